# revision 30
# baseline (speedup 1.0000x reference)
"""Trainium2 Bass kernel for nn_Attention_86199993631321.

Reference computation (B=8, N=128, H=512):
    pair[b,i,j,:] = x[b,i,:] + x[b,j,:]
    out = pair @ W.T + b                # [B, N, N, H]

Algebraic simplification: out[b,i,j,:] = P[b,i,:] + P[b,j,:] where
P = x @ W.T + 0.5*b.  Sharding: data-parallel over batch B.

v4: the device writes a 32-aligned triangle superset of the symmetric
output and the host mirrors the rest.  All broadcast/add/store work is
done in PAIR-sized ops (8 j-columns = 4096 free elements) to amortize
per-op overheads:
  tall pairs (j>=64): one pb / shuffle + one TT + one 8KB-line DMA
  quad pairs (j<64): blocks (t, t+8) stacked at partitions 0/64; one
    shuffle serves 4 blocks; one TT; two merged DMAs
Routes: G (GpSimd partition_broadcast, int32-bitcast), S (DVE
stream_shuffle, int32), PP (PE identity+ones matmuls, ScalarE evict).
bf16 output; host upcasts and mirrors.
"""

import sys

if "/opt/trn_rl_repo" not in sys.path:
    sys.path.insert(0, "/opt/trn_rl_repo")

import numpy as np

B, N, H = 8, 128, 512
NCORES = 8
KC = H // 128
HN = N // 2
JB = 4
WXW = N + H + 128
AUXW = 384

# tall pairs m=0..7 cover blocks (16+2m, 17+2m); heights 96 (m<4) / 128
ROUTES_TALL_PAIRS = ["G", "G", "G", "G", "G", "S", "PP", "PP"]
# quad pairs q=0..3 cover tiles (2q, 2q+1) = blocks (2q, 2q+1) at i<32
# and blocks (2q+8, 2q+9) at i<64 (partitions 64+)
ROUTES_QUAD_PAIRS = ["S", "S", "S", "S"]

_BUILT = {}


def _build_nc():
    import concourse.bass as bass
    import concourse.bacc as bacc
    import concourse.tile as tile
    from concourse import mybir

    f32 = mybir.dt.float32
    bf16 = mybir.dt.bfloat16
    i32 = mybir.dt.int32
    ADD = mybir.AluOpType.add
    COPY = mybir.ActivationFunctionType.Copy

    nc = bacc.Bacc()
    wx_ext = nc.declare_dram_parameter("wx", [H, WXW], bf16, isOutput=False)
    aux_ext = nc.declare_dram_parameter("aux", [128, AUXW], bf16, isOutput=False)
    hb_ext = nc.declare_dram_parameter("halfb", [1, H], bf16, isOutput=False)
    out_ext = nc.declare_dram_parameter("out", [N, N, H], bf16, isOutput=True)

    with tile.TileContext(nc) as tc:
        with (
            tc.tile_pool(name="const", bufs=1) as const,
            tc.tile_pool(name="bcast", bufs=3) as bcast,
            tc.tile_pool(name="outp", bufs=5) as outp,
            tc.tile_pool(name="psum", bufs=2, space="PSUM") as psum,
        ):
            # ---- load packed inputs ----
            wx_sb = const.tile([128, KC, WXW], bf16)
            wx_v = wx_ext.rearrange("(c p) m -> p c m", p=128)
            for c in range(KC):
                eng = nc.sync if c % 2 == 0 else nc.scalar
                eng.dma_start(out=wx_sb[:, c, :], in_=wx_v[:, c, :])
            aux_sb = const.tile([128, AUXW], bf16)
            nc.gpsimd.dma_start(out=aux_sb, in_=aux_ext[:, :])
            hb_sb = const.tile([1, H], bf16)
            nc.gpsimd.dma_start(out=hb_sb, in_=hb_ext[:, :])
            ones_sb = aux_sb[:, 0:128]
            ident_sb = aux_sb[:, 128:256]

            # ---- P = x @ W.T + 0.5*b -> PSUM ----
            ps_proj = psum.tile([128, JB * H], f32, tag="ps", name="proj")
            for c in range(KC):
                nc.tensor.matmul(
                    ps_proj[:, 0:H],
                    wx_sb[:, c, 0:N],
                    wx_sb[:, c, N : N + H],
                    start=(c == 0),
                    stop=False,
                )
            nc.tensor.matmul(
                ps_proj[:, 0:H],
                wx_sb[0:1, 0, N + H : N + H + 128],
                hb_sb,
                start=False,
                stop=True,
            )

            # P_rep8: P replicated 8x along free dim (TT in0 for 4096-wide
            # TTs).  Slots 0-3 from PSUM (ACT/DVE copies), 4-7 via DMA dup.
            P_rep8 = const.tile([128, 8, H], bf16)
            for u in range(JB):
                if u % 2 == 0:
                    nc.scalar.activation(P_rep8[:, u, :], ps_proj[:, 0:H], COPY)
                else:
                    nc.vector.tensor_copy(P_rep8[:, u, :], ps_proj[:, 0:H])
            nc.sync.dma_start(out=P_rep8[:, 4:8, :], in_=P_rep8[:, 0:4, :])
            P0 = P_rep8[:, 0, :]  # [128, 512] view of P

            # chunk0[0, m, :]: 4096-elem chunk of tall PAIR m (blocks
            # 16+2m, 17+2m = P rows 64+8m..64+8m+8), all in partition 0.
            chunk0 = const.tile([1, 8, 2 * JB * H], bf16)
            nc.sync.dma_start(out=chunk0[0:1, :, :], in_=P0[64:128, :])

            # chunk_pp: PE-route chunks, block t (28..31) at partition 96,
            # slot t-28.
            chunk_pp = const.tile([128, JB, JB * H], bf16)
            nc.gpsimd.dma_start(
                out=chunk_pp[96:97, :, :], in_=P0[112:128, :]
            )

            # P_repq8: partition p -> P[p%64], 8 slots (quad TT in0).
            # Reads only slots 0:4 of P_rep8 (no dep on the dup DMA).
            P_repq8 = const.tile([128, 8, H], bf16)
            for n, (pb, sl) in enumerate(((0, 0), (0, 4), (64, 0), (64, 4))):
                eng = nc.gpsimd if n % 2 == 0 else nc.sync
                eng.dma_start(
                    out=P_repq8[pb : pb + 64, sl : sl + 4, :],
                    in_=P_rep8[0:64, 0:4, :],
                )

            # chunkQ2: quad shuffle source.  chunkQ2[32s + q] = pair-chunk
            # of blocks (2q, 2q+1) for s<2, (2q+8, 2q+9) for s>=2.
            chunkQ2 = const.tile([128, 2 * JB * H], bf16)
            for s in range(4):
                lo = 0 if s < 2 else 32
                nc.scalar.dma_start(
                    out=chunkQ2[32 * s : 32 * s + 4, :], in_=P0[lo : lo + 32, :]
                )
                nc.scalar.dma_start(
                    out=chunkQ2[32 * s + 4 : 32 * s + 32, :],
                    in_=P_rep8[0:56, 0:4, :],
                )

            # chunkT2: tall shuffle source (staged late; only the last
            # tall pair shuffles).  chunkT2[32s + m] = pair-chunk m.
            chunkT2 = const.tile([128, 2 * JB * H], bf16)

            def stage_chunkT2():
                for s in range(4):
                    nc.sync.dma_start(
                        out=chunkT2[32 * s : 32 * s + 8, :], in_=P0[64:128, :]
                    )
                    nc.gpsimd.dma_start(
                        out=chunkT2[32 * s + 8 : 32 * s + 32, :],
                        in_=P_rep8[0:48, 0:4, :],
                    )

            def bcast_mms(ps_t, t, pbase, height, stop, start=True):
                s8 = t - 28
                for u in range(JB):
                    nc.tensor.matmul(
                        ps_t[pbase : pbase + height, u * H : (u + 1) * H],
                        ones_sb[96:97, 0:height],
                        chunk_pp[96:97, s8, u * H : (u + 1) * H],
                        start=start,
                        stop=stop,
                        tile_position=(96, pbase),
                        skip_group_check=True,
                    )

            def ident_mms(ps_t, lhsT, rhs):
                for u in range(JB):
                    nc.tensor.matmul(
                        ps_t[:, u * H : (u + 1) * H],
                        lhsT,
                        rhs,
                        start=True,
                        stop=False,
                        skip_group_check=True,
                    )

            dma_eng = [0]

            def out_dma(src_ap, dst_ap):
                # sync twice as often as scalar (scalar also runs ACT work)
                eng = nc.scalar if dma_eng[0] % 3 == 2 else nc.sync
                dma_eng[0] += 1
                eng.dma_start(out=dst_ap, in_=src_ap)

            # ---- tall pairs ----
            def tall_pair(m):
                route = ROUTES_TALL_PAIRS[m]
                bt0 = 16 + 2 * m
                h = 96 if m < 4 else 128
                j0 = JB * bt0
                osb = outp.tile([128, 2 * JB, H], bf16, name="osb")
                if route in ("G", "S"):
                    bc = bcast.tile([128, JB * H], i32, name="bci")
                    if route == "G":
                        nc.gpsimd.partition_broadcast(
                            bc[0:h, :], chunk0[0:1, m, :].bitcast(i32), channels=h
                        )
                    else:
                        nc.vector.stream_shuffle(
                            bc, chunkT2[:, :].bitcast(i32), mask=[m] * 32
                        )
                    bcv = bc.bitcast(bf16).rearrange("p (u h) -> p u h", u=2 * JB)
                    nc.vector.tensor_tensor(
                        out=osb[0:h, :, :],
                        in0=P_rep8[0:h, :, :],
                        in1=bcv[0:h, :, :],
                        op=ADD,
                    )
                else:  # PP: per-block matmuls + per-block eviction
                    for half, bt in enumerate((bt0, bt0 + 1)):
                        ps_t = psum.tile(
                            [128, JB * H], f32, tag="ps", name="pst"
                        )
                        ident_mms(ps_t, ident_sb, P0)
                        bcast_mms(ps_t, bt, 0, h, stop=True, start=False)
                        nc.scalar.activation(
                            osb[0:h, half * JB : (half + 1) * JB, :],
                            ps_t.rearrange("p (u h) -> p u h", u=JB)[0:h, :, :],
                            COPY,
                        )
                out_dma(osb[0:h, :, :], out_ext[0:h, j0 : j0 + 2 * JB, :])

            # ---- quad pairs: one shuffle covers blocks (2q, 2q+1) at
            # partitions <64 and (2q+8, 2q+9) at partitions >=64 ----
            def quad_pair(q):
                osb = outp.tile([128, 2 * JB, H], bf16, name="osb")
                bc = bcast.tile([128, JB * H], i32, name="bci")
                nc.vector.stream_shuffle(
                    bc, chunkQ2[:, :].bitcast(i32), mask=[q] * 32
                )
                bcv = bc.bitcast(bf16).rearrange("p (u h) -> p u h", u=2 * JB)
                nc.vector.tensor_tensor(
                    out=osb, in0=P_repq8, in1=bcv, op=ADD
                )
                j0 = 8 * q
                out_dma(osb[0:32, :, :], out_ext[0:32, j0 : j0 + 8, :])
                out_dma(osb[64:128, :, :], out_ext[0:64, 32 + j0 : 40 + j0, :])

            # PP pairs first (need only P0 + chunk_pp; PE covers the
            # staging window), then G/quad interleaved; the shuffle-only
            # tall pair last, its staging deferred.
            tall_pair(6)
            tall_pair(7)
            tall_pair(0)
            quad_pair(0)
            tall_pair(1)
            quad_pair(1)
            stage_chunkT2()
            tall_pair(2)
            quad_pair(2)
            tall_pair(3)
            quad_pair(3)
            tall_pair(4)
            tall_pair(5)
    nc.compile()
    return nc


def _get_nc():
    if "nc" not in _BUILT:
        _BUILT["nc"] = _build_nc()
    return _BUILT["nc"]


def _make_in_maps(local_feats, W, b):
    import ml_dtypes

    bf = ml_dtypes.bfloat16
    local_feats = np.asarray(local_feats, dtype=np.float32)
    W = np.asarray(W, dtype=np.float32)
    b = np.asarray(b, dtype=np.float32)
    hb = np.ascontiguousarray((0.5 * b).reshape(1, H)).astype(bf)

    aux = np.zeros((128, AUXW), dtype=np.float32)
    aux[:, 0:128] = 1.0
    aux[:, 128:256] = np.eye(128)
    aux_bf = aux.astype(bf)

    base = np.zeros((H, WXW), dtype=np.float32)
    base[:, N : N + H] = W.T
    base[0, N + H :] = 1.0
    in_maps = []
    for c in range(NCORES):
        wx = base.copy()
        wx[:, :N] = local_feats[c].T
        in_maps.append({"wx": wx.astype(bf), "aux": aux_bf, "halfb": hb})
    return in_maps


def _collect(res):
    outs = []
    for c in range(NCORES):
        o = np.asarray(res.results[c]["out"]).astype(np.float32)
        # written: j>=64 at h=96 (j<96) / 128; j<32 at i<32; 32<=j<64 at
        # i<64.  Mirror the symmetric remainder.
        o[32:64, 0:32, :] = o[0:32, 32:64, :].swapaxes(0, 1)
        o[96:128, 64:96, :] = o[64:96, 96:128, :].swapaxes(0, 1)
        o[64:128, 0:64, :] = o[0:64, 64:128, :].swapaxes(0, 1)
        outs.append(o)
    return np.stack(outs, axis=0)


def kernel(local_feats, W, b):
    from concourse.bass_utils import run_bass_kernel_spmd

    nc = _get_nc()
    in_maps = _make_in_maps(local_feats, W, b)
    res = run_bass_kernel_spmd(nc, in_maps, core_ids=list(range(NCORES)))
    return _collect(res)


def run_profiled(local_feats, W, b, **trace_kwargs):
    from concourse.bass_utils import run_bass_kernel_spmd

    nc = _get_nc()
    in_maps = _make_in_maps(local_feats, W, b)
    res = run_bass_kernel_spmd(
        nc, in_maps, core_ids=list(range(NCORES)), trace=True, **trace_kwargs
    )
    return _collect(res), res


# revision 31
# speedup vs baseline: 1.1468x; 1.1468x over previous
"""Trainium2 Bass kernel for nn_Attention_86199993631321.

Reference computation (B=8, N=128, H=512):
    pair[b,i,j,:] = x[b,i,:] + x[b,j,:]
    out = pair @ W.T + b                # [B, N, N, H]

Algebraic simplification: out[b,i,j,:] = P[b,i,:] + P[b,j,:] where
P = x @ W.T + 0.5*b.  Sharding: data-parallel over batch B.

v4: the device writes a 32-aligned triangle superset of the symmetric
output and the host mirrors the rest.  All broadcast/add/store work is
done in PAIR-sized ops (8 j-columns = 4096 free elements) to amortize
per-op overheads:
  tall pairs (j>=64): one pb / shuffle + one TT + one 8KB-line DMA
  quad pairs (j<64): blocks (t, t+8) stacked at partitions 0/64; one
    shuffle serves 4 blocks; one TT; two merged DMAs
Routes: G (GpSimd partition_broadcast, int32-bitcast), S (DVE
stream_shuffle, int32), PP (PE identity+ones matmuls, ScalarE evict).
bf16 output; host upcasts and mirrors.
"""

import sys

if "/opt/trn_rl_repo" not in sys.path:
    sys.path.insert(0, "/opt/trn_rl_repo")

import numpy as np

B, N, H = 8, 128, 512
NCORES = 8
KC = H // 128
HN = N // 2
JB = 4
WXW = N + H + 128
AUXW = 384

# tall pairs m=0..7 cover blocks (16+2m, 17+2m); heights 96 (m<4) / 128
ROUTES_TALL_PAIRS = ["G", "G", "G", "G", "G", "PP", "PP", "PP"]
# quad pairs q=0..3 cover tiles (2q, 2q+1) = blocks (2q, 2q+1) at i<32
# and blocks (2q+8, 2q+9) at i<64 (partitions 64+)
ROUTES_QUAD_PAIRS = ["S", "S", "S", "S"]

_BUILT = {}


def _build_nc():
    import concourse.bass as bass
    import concourse.bacc as bacc
    import concourse.tile as tile
    from concourse import mybir

    f32 = mybir.dt.float32
    bf16 = mybir.dt.bfloat16
    i32 = mybir.dt.int32
    ADD = mybir.AluOpType.add
    COPY = mybir.ActivationFunctionType.Copy

    nc = bacc.Bacc()
    wx_ext = nc.declare_dram_parameter("wx", [H, WXW], bf16, isOutput=False)
    aux_ext = nc.declare_dram_parameter("aux", [128, AUXW], bf16, isOutput=False)
    hb_ext = nc.declare_dram_parameter("halfb", [1, H], bf16, isOutput=False)
    out_ext = nc.declare_dram_parameter("out", [N, N, H], bf16, isOutput=True)

    with tile.TileContext(nc) as tc:
        with (
            tc.tile_pool(name="const", bufs=1) as const,
            tc.tile_pool(name="bcast", bufs=5) as bcast,
            tc.tile_pool(name="outp", bufs=5) as outp,
            tc.tile_pool(name="psum", bufs=2, space="PSUM") as psum,
        ):
            # ---- load packed inputs ----
            wx_sb = const.tile([128, KC, WXW], bf16)
            wx_v = wx_ext.rearrange("(c p) m -> p c m", p=128)
            for c in range(KC):
                eng = nc.sync if c % 2 == 0 else nc.scalar
                eng.dma_start(out=wx_sb[:, c, :], in_=wx_v[:, c, :])
            aux_sb = const.tile([128, AUXW], bf16)
            nc.gpsimd.dma_start(out=aux_sb, in_=aux_ext[:, :])
            hb_sb = const.tile([1, H], bf16)
            nc.gpsimd.dma_start(out=hb_sb, in_=hb_ext[:, :])
            ones_sb = aux_sb[:, 0:128]
            ident_sb = aux_sb[:, 128:256]

            # ---- P = x @ W.T + 0.5*b -> PSUM ----
            ps_proj = psum.tile([128, JB * H], f32, tag="ps", name="proj")
            for c in range(KC):
                nc.tensor.matmul(
                    ps_proj[:, 0:H],
                    wx_sb[:, c, 0:N],
                    wx_sb[:, c, N : N + H],
                    start=(c == 0),
                    stop=False,
                )
            nc.tensor.matmul(
                ps_proj[:, 0:H],
                wx_sb[0:1, 0, N + H : N + H + 128],
                hb_sb,
                start=False,
                stop=True,
            )

            # P_rep8: P replicated 8x along free dim (TT in0 for 4096-wide
            # TTs).  Slots 0-3 from PSUM (ACT/DVE copies), 4-7 via DMA dup.
            P_rep8 = const.tile([128, 8, H], bf16)
            for u in range(JB):
                if u % 2 == 0:
                    nc.scalar.activation(P_rep8[:, u, :], ps_proj[:, 0:H], COPY)
                else:
                    nc.vector.tensor_copy(P_rep8[:, u, :], ps_proj[:, 0:H])
            nc.sync.dma_start(
                out=P_rep8[0:64, 4:8, :], in_=P_rep8[0:64, 0:4, :]
            )
            nc.scalar.dma_start(
                out=P_rep8[64:128, 4:8, :], in_=P_rep8[64:128, 0:4, :]
            )
            P0 = P_rep8[:, 0, :]  # [128, 512] view of P

            # chunk0[0, m, :]: 4096-elem chunk of tall PAIR m (blocks
            # 16+2m, 17+2m = P rows 64+8m..64+8m+8), all in partition 0.
            chunk0 = const.tile([1, 5, 2 * JB * H], bf16)
            nc.gpsimd.dma_start(out=chunk0[0:1, :, :], in_=P0[64:104, :])

            # chunk_pp: PE-route chunks, block t (26..31) at partition 96,
            # slot t-26.
            chunk_pp = const.tile([128, 6, JB * H], bf16)
            nc.gpsimd.dma_start(
                out=chunk_pp[96:97, :, :], in_=P0[104:128, :]
            )

            # P_repq8: partition p -> P[p%64], 8 slots (quad TT in0).
            # Reads only slots 0:4 of P_rep8 (no dep on the dup DMA).
            P_repq8 = const.tile([128, 8, H], bf16)
            engs = (nc.gpsimd, nc.sync, nc.scalar, nc.gpsimd)
            for n, (pb, sl) in enumerate(((0, 0), (0, 4), (64, 0), (64, 4))):
                engs[n].dma_start(
                    out=P_repq8[pb : pb + 64, sl : sl + 4, :],
                    in_=P_rep8[0:64, 0:4, :],
                )

            # chunkQ2: quad shuffle source.  chunkQ2[32s + q] = pair-chunk
            # of blocks (2q, 2q+1) for s<2, (2q+8, 2q+9) for s>=2.
            chunkQ2 = const.tile([128, 2 * JB * H], bf16)
            for s in range(4):
                lo = 0 if s < 2 else 32
                eng = nc.sync if s % 2 == 0 else nc.scalar
                eng.dma_start(
                    out=chunkQ2[32 * s : 32 * s + 4, :], in_=P0[lo : lo + 32, :]
                )
                eng.dma_start(
                    out=chunkQ2[32 * s + 4 : 32 * s + 32, :],
                    in_=P_rep8[0:56, 0:4, :],
                )

            def bcast_mms(ps_t, t, pbase, height, stop, start=True):
                s8 = t - 26
                for u in range(JB):
                    nc.tensor.matmul(
                        ps_t[pbase : pbase + height, u * H : (u + 1) * H],
                        ones_sb[96:97, 0:height],
                        chunk_pp[96:97, s8, u * H : (u + 1) * H],
                        start=start,
                        stop=stop,
                        tile_position=(96, pbase),
                        skip_group_check=True,
                    )

            def ident_mms(ps_t, lhsT, rhs):
                for u in range(JB):
                    nc.tensor.matmul(
                        ps_t[:, u * H : (u + 1) * H],
                        lhsT,
                        rhs,
                        start=True,
                        stop=False,
                        skip_group_check=True,
                    )

            dma_eng = [0]

            def out_dma(src_ap, dst_ap):
                # sync twice as often as scalar (scalar also runs ACT work)
                eng = nc.scalar if dma_eng[0] % 3 == 2 else nc.sync
                dma_eng[0] += 1
                eng.dma_start(out=dst_ap, in_=src_ap)

            # ---- tall pairs ----
            def tall_pair(m):
                route = ROUTES_TALL_PAIRS[m]
                bt0 = 16 + 2 * m
                h = 96 if m < 4 else 128
                j0 = JB * bt0
                osb = outp.tile([128, 2 * JB, H], bf16, name="osb")
                if route in ("G", "S"):
                    bc = bcast.tile([128, JB * H], i32, name="bci")
                    if route == "G":
                        nc.gpsimd.partition_broadcast(
                            bc[0:h, :], chunk0[0:1, m, :].bitcast(i32), channels=h
                        )
                    else:
                        raise AssertionError("tall S route removed")
                    bcv = bc.bitcast(bf16).rearrange("p (u h) -> p u h", u=2 * JB)
                    nc.vector.tensor_tensor(
                        out=osb[0:h, :, :],
                        in0=P_rep8[0:h, :, :],
                        in1=bcv[0:h, :, :],
                        op=ADD,
                    )
                else:  # PP: per-block matmuls + per-block eviction
                    for half, bt in enumerate((bt0, bt0 + 1)):
                        ps_t = psum.tile(
                            [128, JB * H], f32, tag="ps", name="pst"
                        )
                        ident_mms(ps_t, ident_sb, P0)
                        bcast_mms(ps_t, bt, 0, h, stop=True, start=False)
                        nc.scalar.activation(
                            osb[0:h, half * JB : (half + 1) * JB, :],
                            ps_t.rearrange("p (u h) -> p u h", u=JB)[0:h, :, :],
                            COPY,
                        )
                out_dma(osb[0:h, :, :], out_ext[0:h, j0 : j0 + 2 * JB, :])

            # ---- quad pairs: one shuffle covers blocks (2q, 2q+1) at
            # partitions <64 and (2q+8, 2q+9) at partitions >=64 ----
            def quad_pair(q):
                osb = outp.tile([128, 2 * JB, H], bf16, name="osb")
                bc = bcast.tile([128, JB * H], i32, name="bci")
                nc.vector.stream_shuffle(
                    bc, chunkQ2[:, :].bitcast(i32), mask=[q] * 32
                )
                bcv = bc.bitcast(bf16).rearrange("p (u h) -> p u h", u=2 * JB)
                nc.vector.tensor_tensor(
                    out=osb, in0=P_repq8, in1=bcv, op=ADD
                )
                j0 = 8 * q
                out_dma(osb[0:32, :, :], out_ext[0:32, j0 : j0 + 8, :])
                out_dma(osb[64:128, :, :], out_ext[0:64, 32 + j0 : 40 + j0, :])

            # PP pairs first (PE covers the staging window), then the
            # quad shuffles (independent of pb), then the G pairs.
            tall_pair(5)
            tall_pair(6)
            tall_pair(7)
            quad_pair(0)
            quad_pair(1)
            tall_pair(0)
            quad_pair(2)
            tall_pair(1)
            quad_pair(3)
            tall_pair(2)
            tall_pair(3)
            tall_pair(4)
    nc.compile()
    return nc


def _get_nc():
    if "nc" not in _BUILT:
        _BUILT["nc"] = _build_nc()
    return _BUILT["nc"]


def _make_in_maps(local_feats, W, b):
    import ml_dtypes

    bf = ml_dtypes.bfloat16
    local_feats = np.asarray(local_feats, dtype=np.float32)
    W = np.asarray(W, dtype=np.float32)
    b = np.asarray(b, dtype=np.float32)
    hb = np.ascontiguousarray((0.5 * b).reshape(1, H)).astype(bf)

    aux = np.zeros((128, AUXW), dtype=np.float32)
    aux[:, 0:128] = 1.0
    aux[:, 128:256] = np.eye(128)
    aux_bf = aux.astype(bf)

    base = np.zeros((H, WXW), dtype=np.float32)
    base[:, N : N + H] = W.T
    base[0, N + H :] = 1.0
    in_maps = []
    for c in range(NCORES):
        wx = base.copy()
        wx[:, :N] = local_feats[c].T
        in_maps.append({"wx": wx.astype(bf), "aux": aux_bf, "halfb": hb})
    return in_maps


def _collect(res):
    outs = []
    for c in range(NCORES):
        o = np.asarray(res.results[c]["out"]).astype(np.float32)
        # written: j>=64 at h=96 (j<96) / 128; j<32 at i<32; 32<=j<64 at
        # i<64.  Mirror the symmetric remainder.
        o[32:64, 0:32, :] = o[0:32, 32:64, :].swapaxes(0, 1)
        o[96:128, 64:96, :] = o[64:96, 96:128, :].swapaxes(0, 1)
        o[64:128, 0:64, :] = o[0:64, 64:128, :].swapaxes(0, 1)
        outs.append(o)
    return np.stack(outs, axis=0)


def kernel(local_feats, W, b):
    from concourse.bass_utils import run_bass_kernel_spmd

    nc = _get_nc()
    in_maps = _make_in_maps(local_feats, W, b)
    res = run_bass_kernel_spmd(nc, in_maps, core_ids=list(range(NCORES)))
    return _collect(res)


def run_profiled(local_feats, W, b, **trace_kwargs):
    from concourse.bass_utils import run_bass_kernel_spmd

    nc = _get_nc()
    in_maps = _make_in_maps(local_feats, W, b)
    res = run_bass_kernel_spmd(
        nc, in_maps, core_ids=list(range(NCORES)), trace=True, **trace_kwargs
    )
    return _collect(res), res
